# revision 1
# baseline (speedup 1.0000x reference)
"""CQT layer kernel for Trainium2 (8 NeuronCores, SPMD).

The strided conv (hop 128 == PE contraction tile) is a chunked matmul:
  out[c, b, t] = sum_k  W[c, 128k:128k+128] . xT_b[:, t+k]
where xT_b is the zero-padded audio reshaped to [128, 672] (a free reshape,
because hop == 128).  Each core holds 128 of the 1056 output channels as the
stationary operand (full PE array) and streams the frame columns; the 32
leftover channels are split across cores by chunk range and summed on host.
Magnitude + power_to_db run on host, with an exact fp64 recompute of the few
near-silent bins where reduced-precision matmul error would be audible in dB.

Self-contained: only needs numpy + the concourse toolchain at /opt/trn_rl_repo.
"""
import os
import sys

sys.path.insert(0, "/opt/trn_rl_repo")
import numpy as np

# ---- problem constants (hardcoded from the CQT layer spec) ----
B = 2
AUDIO_LEN = 22016
N_BINS = 528
NCH = 2 * N_BINS          # 1056 conv channels (re, im)
HOP = 128
FRAMES = 173
AMIN = 1e-10
TOP_DB = 80.0

K = 128                   # PE contraction tile == HOP
NCHUNK = 499              # ceil(L / 128); holds for L in [63745, 63872]
LPAD = NCHUNK * K         # 63872
NT = 174                  # frames padded to even (fp32r needs even free dims)
NROW = NCHUNK + NT - 1    # 672 columns of xT per batch
N_CORES = 8
MAIN_CH = 128             # stationary channels per core
TAIL_CH = NCH - N_CORES * MAIN_CH   # 32
TPC = 63                  # tail chunks per core (8*63 = 504 >= 499)
TCOLS = TPC + NT - 1      # 236 xT columns each core needs for its tail window
GROUP = int(os.environ.get("CQT_GROUP", "32"))  # weight chunks per DMA group

DTYPE = os.environ.get("CQT_DTYPE", "float16")  # float16 | float32r
# device matmul relative error (vs conv rms); drives the host refinement
# threshold for near-silent bins.  abs_err ~= eps * rms(conv) because the
# per-product rounding errors accumulate like the products themselves.
_CONV_EPS = {"float16": 1e-3, "float32r": 5e-4, "bfloat16": 5e-3}
DB_ERR_TARGET = 0.02      # refine bins whose worst-case dB error exceeds this

_prog_cache = {}


def _np_cast(a):
    if DTYPE == "float16":
        return a.astype(np.float16)
    if DTYPE == "bfloat16":
        import ml_dtypes
        return a.astype(ml_dtypes.bfloat16)
    return a  # float32r: raw fp32 bits


def _build_program():
    from concourse import bacc, mybir
    from concourse.tile import TileContext

    dt = mybir.dt
    DT = getattr(dt, DTYPE)

    nc = bacc.Bacc(None, target_bir_lowering=False)
    xt_p = nc.declare_dram_parameter("xt", [K, B * NROW], DT, isOutput=False)
    xtl_p = nc.declare_dram_parameter("xtl", [K, B * TCOLS], DT, isOutput=False)
    wm_p = nc.declare_dram_parameter("wm", [K, NCHUNK * MAIN_CH], DT, isOutput=False)
    wt_p = nc.declare_dram_parameter("wt", [K, TPC * TAIL_CH], DT, isOutput=False)
    om_p = nc.declare_dram_parameter("om", [MAIN_CH, B * NT], dt.float32, isOutput=True)
    ot_p = nc.declare_dram_parameter("ot", [MAIN_CH, B * NT], dt.float32, isOutput=True)

    # main weight groups: small first so the PE starts streaming early,
    # then 32-chunk (1 MB) steady-state DMAs
    groups = []
    k0 = 0
    ramp = [int(v) for v in os.environ.get("CQT_RAMP", "4,8,16").split(",") if v]
    for g in ramp:
        groups.append((k0, g))
        k0 += g
    while k0 < NCHUNK:
        cnt = min(GROUP, NCHUNK - k0)
        groups.append((k0, cnt))
        k0 += cnt
    XP2_AFTER = 3          # second half of x frames rides behind early groups
    TAIL_INPUT_AFTER = 6   # issue tail-input DMAs once main supply is ahead
    TAIL_MM_AFTER = 8      # run tail matmuls mid-stream; epilogue overlaps main
    N_WARM = int(os.environ.get("CQT_WARM", "10"))  # HAM warm-up matmuls

    with TileContext(nc) as tc:
        with (
            tc.tile_pool(name="stat", bufs=1) as stat,
            tc.tile_pool(name="wpool", bufs=4) as wpool,
            tc.tile_pool(name="opool", bufs=1) as opool,
            tc.tile_pool(name="ps", bufs=1, space="PSUM") as ps,
        ):
            # PE warm-up on a memset tile: no DMA dependency, runs during
            # the input-DMA window so HAM reaches 2.4 GHz before real work
            warm_sb = stat.tile([K, B * NT], DT)
            nc.gpsimd.memset(warm_sb[:], 0.0)
            ps_warm = ps.tile([K, B * NT], dt.float32)
            for _ in range(N_WARM):
                nc.tensor.matmul(ps_warm[:], warm_sb[:, :K], warm_sb[:],
                                 start=True, stop=True)

            # critical-path inputs: first half of the (t,b)-interleaved x
            # frames (enough for chunks 0..162), then ramped weight groups;
            # the rest of x and the tail inputs ride behind.
            xt_sb = stat.tile([K, B * NROW], DT)
            nc.sync.dma_start(xt_sb[:, :NROW], xt_p[:, :NROW])
            xtl_sb = stat.tile([K, B * TCOLS], DT)
            wt_sb = stat.tile([K, TPC * TAIL_CH], DT)
            wgs = []
            for gi, (g0, cnt) in enumerate(groups):
                wg = wpool.tile([K, GROUP * MAIN_CH], DT, tag="wg")
                nc.sync.dma_start(
                    wg[:, :cnt * MAIN_CH],
                    wm_p[:, g0 * MAIN_CH:(g0 + cnt) * MAIN_CH],
                )
                wgs.append(wg)
                if gi == XP2_AFTER:
                    nc.sync.dma_start(xt_sb[:, NROW:], xt_p[:, NROW:])
                if gi == TAIL_INPUT_AFTER:
                    nc.sync.dma_start(xtl_sb[:], xtl_p[:])
                    nc.sync.dma_start(wt_sb[:], wt_p[:])

            x3 = xt_sb[:].rearrange("p (t b) -> p t b", b=B)
            xl3 = xtl_sb[:].rearrange("p (t b) -> p t b", b=B)

            ps_main = ps.tile([MAIN_CH, B * NT], dt.float32)
            pm3 = ps_main[:].rearrange("p (t b) -> p t b", b=B)
            ps_tg = []
            for g in range(4):
                ptile = ps.tile([MAIN_CH, B * NT], dt.float32, tag=f"pt{g}", name=f"pt{g}")
                ps_tg.append(ptile)
            pt3g = [p[32 * g:32 * (g + 1), :].rearrange("p (t b) -> p t b", b=B)
                    for g, p in enumerate(ps_tg)]
            ot_sb = opool.tile([MAIN_CH, B * NT], dt.float32)

            def tail_block():
                # 32 channels x 63 chunks, 4-way column-tiled: four M=32
                # matmuls run concurrently in distinct PE column groups,
                # each accumulating in its own PSUM bank at partitions
                # [32g, 32g+32); host sums the four partials.
                for j in range(TPC):
                    g = j % 4
                    nc.tensor.matmul(
                        pt3g[g],
                        wt_sb[:, j * TAIL_CH:(j + 1) * TAIL_CH],
                        xl3[:, j:j + NT, :],
                        start=(j < 4),
                        stop=(j + 4 >= TPC),
                        tile_position=(0, 32 * g),
                    )
                for g in range(4):
                    nc.vector.tensor_copy(
                        ot_sb[32 * g:32 * (g + 1), :],
                        ps_tg[g][32 * g:32 * (g + 1), :],
                    )
                nc.sync.dma_start(ot_p[:], ot_sb[:])

            # main: 128 stationary channels x 499 chunks, weights streamed
            for gi, ((g0, cnt), wg) in enumerate(zip(groups, wgs)):
                for j in range(cnt):
                    k = g0 + j
                    nc.tensor.matmul(
                        pm3,
                        wg[:, j * MAIN_CH:(j + 1) * MAIN_CH],
                        x3[:, k:k + NT, :],
                        start=(k == 0),
                        stop=(k == NCHUNK - 1),
                    )
                if gi == TAIL_MM_AFTER:
                    tail_block()

            # drain main PSUM in halves so the copy and the out-DMA pipeline
            om_sb = opool.tile([MAIN_CH, B * NT], dt.float32)
            half = B * NT // 2
            for h in range(2):
                sl = slice(h * half, (h + 1) * half)
                nc.vector.tensor_copy(om_sb[:, sl], ps_main[:, sl])
                nc.sync.dma_start(om_p[:, sl], om_sb[:, sl])

    nc.finalize()
    return nc


LAST_RESULTS = None


def kernel(y, kern_r, kern_i):
    global LAST_RESULTS
    from concourse.bass_utils import run_bass_kernel_spmd

    y = np.asarray(y, dtype=np.float32)
    kern_r = np.asarray(kern_r, dtype=np.float32)
    kern_i = np.asarray(kern_i, dtype=np.float32)

    # ---- host prep: weights -> [l, (chunk, ch)] layouts ----
    L_in = kern_r.shape[1]                                 # 63864 from the layer
    pad = L_in // 2
    assert (NCHUNK - 1) * K < L_in <= LPAD, L_in
    W = np.concatenate([kern_r, kern_i], axis=0)          # [1056, L]
    Wp = np.zeros((NCH, LPAD), np.float32)
    Wp[:, :L_in] = W
    Wk = Wp.reshape(NCH, NCHUNK, K)                        # [c, k, l]

    # ---- host prep: audio -> xT [128, per-batch 672 cols] ----
    x_pad = np.zeros((B, NROW * K), np.float32)
    x_pad[:, pad:pad + AUDIO_LEN] = y
    xT = np.ascontiguousarray(x_pad.reshape(B, NROW, K).transpose(0, 2, 1))  # [B,128,672]
    # (t, b)-interleaved columns: col 2t+b = xT[b][:, t]
    xt_il = np.empty((K, B * NROW), np.float32)
    xt_il[:, 0::2] = xT[0]
    xt_il[:, 1::2] = xT[1]
    xt_host = _np_cast(xt_il)                                                # [128, 1344]

    in_maps = []
    for i in range(N_CORES):
        # main weights: channels [128i, 128i+128), all chunks -> [128l, 499*128]
        wm = np.ascontiguousarray(
            Wk[i * MAIN_CH:(i + 1) * MAIN_CH].transpose(2, 1, 0)
        ).reshape(K, NCHUNK * MAIN_CH)
        # tail weights: channels 1024.., chunks [63i, 63i+63) (zero-padded)
        wt = np.zeros((K, TPC, TAIL_CH), np.float32)
        k0, k1 = i * TPC, min((i + 1) * TPC, NCHUNK)
        if k1 > k0:
            wt[:, :k1 - k0, :] = Wk[N_CORES * MAIN_CH:, k0:k1, :].transpose(2, 1, 0)
        wt = wt.reshape(K, TPC * TAIL_CH)
        # tail x window: xT columns [63i, 63i+236), zero-padded past 672
        xtl = np.zeros((B, K, TCOLS), np.float32)
        hi = min(NROW, i * TPC + TCOLS)
        if hi > i * TPC:
            xtl[:, :, :hi - i * TPC] = xT[:, :, i * TPC:hi]
        xtl_host = np.empty((K, B * TCOLS), np.float32)
        xtl_host[:, 0::2] = xtl[0]
        xtl_host[:, 1::2] = xtl[1]
        in_maps.append({
            "xt": xt_host,
            "xtl": _np_cast(np.ascontiguousarray(xtl_host)),
            "wm": _np_cast(wm),
            "wt": _np_cast(np.ascontiguousarray(wt)),
        })

    if DTYPE not in _prog_cache:
        _prog_cache[DTYPE] = _build_program()
    nc = _prog_cache[DTYPE]

    LAST_RESULTS = run_bass_kernel_spmd(
        nc, in_maps, list(range(N_CORES)),
        trace=bool(os.environ.get("CQT_TRACE")),
    )
    results = LAST_RESULTS.results

    # ---- host post: assemble conv, magnitude, power_to_db ----
    conv = np.zeros((NCH, B, FRAMES), np.float32)
    tail = np.zeros((TAIL_CH, B, NT), np.float64)
    for i in range(N_CORES):
        om = results[i]["om"].reshape(MAIN_CH, NT, B).transpose(0, 2, 1)
        conv[i * MAIN_CH:(i + 1) * MAIN_CH] = om[:, :, :FRAMES]
        # [128, NT, B] -> 4 column-group partials at partitions [32g, 32g+32)
        tail += results[i]["ot"].reshape(4, TAIL_CH, NT, B).sum(axis=0).transpose(0, 2, 1)
    conv[N_CORES * MAIN_CH:] = tail[:, :, :FRAMES].astype(np.float32)

    re = conv[:N_BINS].astype(np.float64)                  # [528, B, 173]
    im = conv[N_BINS:].astype(np.float64)
    mag = np.sqrt(re * re + im * im)                       # [528, B, 173]

    # ---- host refinement: exact recompute of near-silent bins ----
    conv_rms = float(np.sqrt(np.mean(mag * mag)))
    err_abs = _CONV_EPS.get(DTYPE, 1e-3) * conv_rms
    thresh = 4.343 * err_abs / DB_ERR_TARGET
    fix = np.argwhere(mag < thresh)                        # rows: (bin, b, t)
    if len(fix):
        W64 = W.astype(np.float64)
        xp64 = x_pad.astype(np.float64)
        for b in range(B):
            sel = fix[fix[:, 1] == b]
            if not len(sel):
                continue
            for t in np.unique(sel[:, 2]):
                bins = sel[sel[:, 2] == t][:, 0]
                win = xp64[b, t * HOP:t * HOP + L_in]
                re[bins, b, t] = W64[bins] @ win
                im[bins, b, t] = W64[bins + N_BINS] @ win
        mag = np.sqrt(re * re + im * im)

    ref = max(mag.max(), AMIN)
    log_spec = 10.0 * np.log10(np.maximum(mag, AMIN)) - 10.0 * np.log10(ref)
    log_spec = np.maximum(log_spec, log_spec.max() - TOP_DB)
    return np.ascontiguousarray(log_spec.transpose(1, 2, 0)).astype(np.float32)



# revision 2
# speedup vs baseline: 1.8137x; 1.8137x over previous
"""CQT layer kernel for Trainium2 (8 NeuronCores, SPMD) — sparse band version.

The CQT filterbank is ~82% zeros: bin k's filter has a centered support of
Nk ~ 63864 * 2^(-k/66) samples.  We exploit this at 16-bin granularity:
group g = bins [16g, 16g+16) x {re, im} = 32 channels whose joint support
spans chunks [c0(g), c1(g)) of the 128-sample contraction grid (hop == 128,
so audio reshaped [128, cols] makes the strided conv a chunked matmul).

Each (group, chunk) unit is a [128k x 32ch] stationary matmul against 348
moving columns (174 frames x 2 batch, (t,b)-interleaved), run in 128x32
column-tiled PE mode: 4 tiles (lanes) sustain ~4 matmuls per 348 cycles.

SPMD uniformity: all 8 cores run the IDENTICAL program.  Core i takes
chunks c0(g)+i, c0(g)+i+8, ... of every group (stride 8).  The program
addresses x at chunk c0(g)+8t; the per-core "+i" shift is folded into the
DATA by shifting core i's x buffer left by i chunks.  Groups are padded to
T(g) = ceil(n(g)/8) slots per core with zero weights (~4% PE overhead).

Per core: ~420 units => ~16 us PE, ~3.5 MB weights + ~1.5 MB partial-out
DMA, overlapped.  Host sums the per-core 32-row partials per group, then
magnitude + power_to_db with an exact fp64 recompute of near-silent bins.

Self-contained: only needs numpy + the concourse toolchain at /opt/trn_rl_repo.
"""
import os
import sys

sys.path.insert(0, "/opt/trn_rl_repo")
import numpy as np

# ---- problem constants (hardcoded from the CQT layer spec) ----
B = 2
AUDIO_LEN = 22016
N_BINS = 528
NCH = 2 * N_BINS          # 1056 conv channels (re, im)
HOP = 128
FRAMES = 173
AMIN = 1e-10
TOP_DB = 80.0

K = 128                   # PE contraction tile == HOP
NT = 174                  # frames padded to even
NF = B * NT               # 348 moving columns per matmul
GB = 16                   # bins per group
NG = N_BINS // GB         # 33 groups
CH = 2 * GB               # 32 channels per group (re+im)
N_CORES = 8
NCHUNK = 499              # ceil(L / 128) for L in (63744, 63872]
XCOLS = 688               # x chunk-columns held on device (>= 499+7+174)
GROUP = int(os.environ.get("CQT_GROUP", "32"))   # weight slots per DMA group
WARM_ROUNDS = int(os.environ.get("CQT_WARM", "8"))

DB_ERR_TARGET = 0.02      # refine bins whose worst-case dB error exceeds this
CONV_EPS = 1e-3           # fp16 device matmul relative error vs conv rms

_prog_cache = {}


def _schedule(T):
    """LPT lane assignment + round-robin issue order.

    Returns (issue, gslots) where issue[s] = (lane, g, t) in emission order
    and gslots[g] = [slot index for t = 0..T[g])].
    """
    order = sorted(range(len(T)), key=lambda g: -T[g])
    lanes = [[] for _ in range(4)]
    loads = [0] * 4
    for g in order:
        L = loads.index(min(loads))
        lanes[L].append(g)
        loads[L] += T[g]
    ptr = [0] * 4            # position in lanes[L]
    tcur = [0] * 4           # step within current group
    issue = []
    gslots = [[None] * T[g] for g in range(len(T))]
    done = 0
    total = sum(T)
    while done < total:
        for L in range(4):
            if ptr[L] >= len(lanes[L]):
                continue
            g = lanes[L][ptr[L]]
            t = tcur[L]
            gslots[g][t] = len(issue)
            issue.append((L, g, t))
            done += 1
            tcur[L] += 1
            if tcur[L] >= T[g]:
                ptr[L] += 1
                tcur[L] = 0
    return issue, gslots


def _build_program(c0, T, issue):
    from concourse import bacc, mybir
    from concourse.tile import TileContext

    dt = mybir.dt
    S = len(issue)
    NGR = len(c0)

    nc = bacc.Bacc(None, target_bir_lowering=False)
    xt_p = nc.declare_dram_parameter("xt", [K, B * XCOLS], dt.float16, isOutput=False)
    wm_p = nc.declare_dram_parameter("wm", [K, S * CH], dt.float16, isOutput=False)
    om_p = nc.declare_dram_parameter("om", [CH, NGR * NF], dt.float32, isOutput=True)

    # weight DMA groups: small first so the PE starts streaming early
    groups = []
    k0 = 0
    for gsz in (4, 8, 16):
        groups.append((k0, gsz))
        k0 += gsz
    while k0 < S:
        cnt = min(GROUP, S - k0)
        groups.append((k0, cnt))
        k0 += cnt

    with TileContext(nc) as tc:
        with (
            tc.tile_pool(name="stat", bufs=1) as stat,
            tc.tile_pool(name="wpool", bufs=4) as wpool,
            tc.tile_pool(name="epool", bufs=2) as epool,
            tc.tile_pool(name="ps", bufs=1, space="PSUM") as ps,
        ):
            # per-lane double-buffered PSUM accumulators
            pst = [[ps.tile([K, NF], dt.float32, tag=f"ps{L}{j}", name=f"ps{L}{j}")
                    for j in range(2)] for L in range(4)]

            # PE warm-up in 128x32 col-tiled mode on a memset tile (no DMA
            # dependency) so the HAM p-state ramps during the input DMAs
            warm_sb = stat.tile([K, NF], dt.float16)
            nc.gpsimd.memset(warm_sb[:], 0.0)
            for r in range(WARM_ROUNDS):
                for L in range(4):
                    nc.tensor.matmul(
                        pst[L][0][32 * L:32 * (L + 1), :],
                        warm_sb[:, :CH], warm_sb[:],
                        start=True, stop=True, tile_position=(0, 32 * L),
                    )

            xt_sb = stat.tile([K, B * XCOLS], dt.float16)
            nc.sync.dma_start(xt_sb[:], xt_p[:])
            x3 = xt_sb[:].rearrange("p (c b) -> p c b", b=B)

            wgs = []
            for (g0, cnt) in groups:
                wg = wpool.tile([K, GROUP * CH], dt.float16, tag="wg")
                nc.sync.dma_start(
                    wg[:, :cnt * CH],
                    wm_p[:, g0 * CH:(g0 + cnt) * CH],
                )
                wgs.append(wg)

            # main stream: issue-ordered col-tiled matmuls, 4 lanes
            pcur = [0] * 4      # psum buffer index per lane
            evict_n = 0
            gi = 0              # current weight DMA group
            for s, (L, g, t) in enumerate(issue):
                g0, cnt = groups[gi]
                if s >= g0 + cnt:
                    gi += 1
                    g0, cnt = groups[gi]
                wg = wgs[gi]
                j = s - g0
                ptile = pst[L][pcur[L]]
                psl = ptile[32 * L:32 * (L + 1), :]
                cprog = c0[g] + 8 * t
                nc.tensor.matmul(
                    psl,
                    wg[:, j * CH:(j + 1) * CH],
                    x3[:, cprog:cprog + NT, :],
                    start=(t == 0),
                    stop=(t == T[g] - 1),
                    tile_position=(0, 32 * L),
                )
                if t == T[g] - 1:
                    # evict the finished group's partial and flip psum buffer
                    ev = epool.tile([K, NF], dt.float32, tag=f"ev{L}")
                    if evict_n % 2 == 0:
                        nc.vector.tensor_copy(ev[32 * L:32 * (L + 1), :], psl)
                    else:
                        nc.scalar.activation(
                            ev[32 * L:32 * (L + 1), :], psl,
                            mybir.ActivationFunctionType.Copy,
                        )
                    evict_n += 1
                    nc.sync.dma_start(
                        om_p[:, g * NF:(g + 1) * NF],
                        ev[32 * L:32 * (L + 1), :],
                    )
                    pcur[L] ^= 1

    nc.finalize()
    return nc


LAST_RESULTS = None


def kernel(y, kern_r, kern_i):
    global LAST_RESULTS
    from concourse.bass_utils import run_bass_kernel_spmd

    y = np.asarray(y, dtype=np.float32)
    kern_r = np.asarray(kern_r, dtype=np.float32)
    kern_i = np.asarray(kern_i, dtype=np.float32)

    L_in = kern_r.shape[1]
    pad = L_in // 2
    W = np.concatenate([kern_r, kern_i], axis=0)           # [1056, L]
    LPAD = NCHUNK * K
    assert L_in <= LPAD, L_in
    Wp = np.zeros((NCH, LPAD), np.float32)
    Wp[:, :L_in] = W
    W3 = Wp.reshape(NCH, NCHUNK, K)                        # [ch, chunk, 128]

    # ---- per-group chunk support from the actual kernel arrays ----
    nz = np.abs(W) > 0
    any_nz = nz.any(axis=1)
    lo_k = np.where(any_nz, nz.argmax(axis=1), 0)
    hi_k = np.where(any_nz, L_in - nz[:, ::-1].argmax(axis=1), 1)
    chs = [np.r_[GB * g:GB * (g + 1), N_BINS + GB * g:N_BINS + GB * (g + 1)]
           for g in range(NG)]
    c0, n = [], []
    for g in range(NG):
        lo = int(lo_k[chs[g]].min())
        hi = int(hi_k[chs[g]].max())
        a = lo // K
        b = min(-(-hi // K), NCHUNK)
        c0.append(a)
        n.append(max(b - a, 1))
    T = [-(-ng // N_CORES) for ng in n]                    # slots per core
    issue, gslots = _schedule(T)
    S = len(issue)

    # ---- host prep: per-core x buffers (global shift by core id) ----
    XF = XCOLS + N_CORES
    x_pad = np.zeros((B, XF * K), np.float32)
    x_pad[:, pad:pad + AUDIO_LEN] = y
    xT = x_pad.reshape(B, XF, K).transpose(0, 2, 1)        # [B, 128, XF]
    xt16 = np.ascontiguousarray(xT).astype(np.float16)

    # ---- host prep: per-core weight streams in issue order ----
    in_maps = []
    for i in range(N_CORES):
        wm = np.zeros((S, CH, K), np.float32)
        for g in range(NG):
            tmax = -(-(n[g] - i) // N_CORES) if n[g] > i else 0
            if tmax <= 0:
                continue
            cs = c0[g] + i + N_CORES * np.arange(tmax)
            sl = np.asarray(gslots[g][:tmax])
            wm[sl] = W3[chs[g]][:, cs, :].transpose(1, 0, 2)
        wm_host = np.ascontiguousarray(
            wm.transpose(2, 0, 1).reshape(K, S * CH)).astype(np.float16)
        xt_i = np.zeros((K, B * XCOLS), np.float16)
        xt_i[:, 0::2] = xt16[0, :, i:i + XCOLS]
        xt_i[:, 1::2] = xt16[1, :, i:i + XCOLS]
        in_maps.append({"xt": np.ascontiguousarray(xt_i), "wm": wm_host})

    key = (tuple(c0), tuple(T))
    if key not in _prog_cache:
        _prog_cache[key] = _build_program(c0, T, issue)
    nc = _prog_cache[key]

    LAST_RESULTS = run_bass_kernel_spmd(
        nc, in_maps, list(range(N_CORES)),
        trace=bool(os.environ.get("CQT_TRACE")),
    )
    results = LAST_RESULTS.results

    # ---- host post: assemble conv from per-core group partials ----
    conv = np.zeros((NCH, B, NT), np.float64)
    for i in range(N_CORES):
        om = results[i]["om"].reshape(CH, NG, NT, B)       # cols = (g, t, b)
        for g in range(NG):
            conv[chs[g]] += om[:, g].transpose(0, 2, 1)
    conv = conv[:, :, :FRAMES]

    re = conv[:N_BINS]
    im = conv[N_BINS:]
    mag = np.sqrt(re * re + im * im)                       # [528, B, 173]

    # ---- host refinement: exact recompute of near-silent bins ----
    conv_rms = float(np.sqrt(np.mean(mag * mag)))
    thresh = 4.343 * CONV_EPS * conv_rms / DB_ERR_TARGET
    fix = np.argwhere(mag < thresh)                        # rows: (bin, b, t)
    if len(fix):
        W64 = W.astype(np.float64)
        xp64 = np.zeros((B, 2 * pad + AUDIO_LEN), np.float64)
        xp64[:, pad:pad + AUDIO_LEN] = y
        for b in range(B):
            sel = fix[fix[:, 1] == b]
            if not len(sel):
                continue
            for t in np.unique(sel[:, 2]):
                bins = sel[sel[:, 2] == t][:, 0]
                win = xp64[b, t * HOP:t * HOP + L_in]
                re[bins, b, t] = W64[bins] @ win
                im[bins, b, t] = W64[bins + N_BINS] @ win
        mag = np.sqrt(re * re + im * im)

    ref = max(mag.max(), AMIN)
    log_spec = 10.0 * np.log10(np.maximum(mag, AMIN)) - 10.0 * np.log10(ref)
    log_spec = np.maximum(log_spec, log_spec.max() - TOP_DB)
    return np.ascontiguousarray(log_spec.transpose(1, 2, 0)).astype(np.float32)
